# revision 1
# baseline (speedup 1.0000x reference)
"""Trainium2 Bass kernel for a Mix9Net-style directional CNN.

Network (per image, 4 directions d with unit vectors u_d):
  xs[d] = silu(dconv3(x, w_d0, b_d0, u_d))                      # Cin=2 -> 128
  4x DirectionalConvResBlock:
      t = silu(dconv3(xs[d], res_wd[l], res_bd[l], u_d))        # 128 -> 128
      t = silu(conv1x1(t, res_w1[l], res_b1[l]))                # 128 -> 128
      xs[d] = t + xs[d]
  Conv0dResBlock:
      xs[d] = silu(conv1x1(silu(conv1x1(xs[d], c0_w1, c0_b1)), c0_w2, c0_b2)) + xs[d]
  out[d] = conv1x1(xs[d], wf, bf)                               # 128 -> 64
Output stacked: [B, 4, 64, 15, 15].

Mapping: channels (128) on SBUF partitions, pixels on the free dim.
Activations live in a padded flat layout with 16-element row pitch
(15 data cols + 1 zero pad col) and 16 rows per image block (15 data
rows + 1 zero pad row), so a directional shift by (dy,dx) is a flat AP
offset of dy*16+dx and the 3-tap directional convs become 3 shifted
matmuls accumulating in PSUM.  Matmuls stream full 16-wide rows — the
fp32r ISA path requires an even innermost element count — so each PSUM
result carries one garbage column per row, stripped on the ScalarE /
VectorE side (engine APs allow odd innermost counts).

All four directions share each layer's weights and are processed in
lockstep: each layer step does 4 matmuls (one per direction) into the 4
PSUM banks of one [128, 4, 512] tile, then a single ScalarE silu+bias
drains all 4 banks (big ACT ops amortize the ~240ns per-op overhead).

Matmuls run as float32r (fp32 data, full PE rate at N >= 256).  The
final 64-channel conv is padded to 128 outputs (fp32r requires full
128-column tiling).

Sharding: pure data parallel, batch 1024 -> 128 images on each of 8 cores.
"""

import numpy as np

import concourse.bacc as bacc
import concourse.tile as tile
from concourse import mybir
from concourse import bass_utils

F32 = mybir.dt.float32
SILU = mybir.ActivationFunctionType.Silu

# geometry
H = 15
RP = 16                     # row pitch: 15 data cols + 1 zero pad col
IMGB = RP * RP              # 256: 15 data rows + 1 zero pad row
GROUP = 2                   # images per matmul
PAY = GROUP * IMGB          # 512
GUARD = 32                  # flat guard so shifted reads stay in bounds
XLEN = GUARD + PAY + GUARD  # 576
NMM = GROUP * H * RP        # 480 matmul columns (incl. pad cols)
NOUT = GROUP * H * H        # 450 real pixels
NB = 512                    # fp32 elements per PSUM bank
NRES = 4
DM = 128
DOUT = 64
CIN = 2
NCORES = 8
BATCH = 1024
BPC = BATCH // NCORES       # images per core
# directions: horizontal, vertical, main diag, anti diag
DIRS = ((0, 1), (1, 0), (1, 1), (1, -1))
OFFS = [dy * RP + dx for (dy, dx) in DIRS]  # flat offsets: 1, 16, 17, 15


def _blocks(t, d, off):
    """[128, GROUP, 16, 16] block view of direction-d payload shifted by off."""
    v = t[:, d, GUARD + off: GUARD + off + PAY]
    return v.rearrange("p (i r c) -> p i r c", i=GROUP, r=RP, c=RP)


def _rhs(t, d, off):
    """[128, GROUP, 15, 16] matmul rhs: 15 data rows, full 16-wide."""
    return _blocks(t, d, off)[:, :, :H, :]


def _int15(t, d):
    """[128, GROUP, 15, 15] data pixels only."""
    return _blocks(t, d, 0)[:, :, :H, :H]


def _w240(t, d):
    """[128, GROUP, 240] contiguous 15 rows x 16 cols per image."""
    v = t[:, d, GUARD: GUARD + PAY]
    return v.rearrange("p (i q) -> p i q", i=GROUP, q=IMGB)[:, :, :H * RP]


def _strip(ps, d):
    """[P, GROUP, 15, 15] strip of a [P, 4, >=NMM] psum/compact tensor."""
    v = ps[:, d, :NMM].rearrange("p (i r c) -> p i r c", i=GROUP, r=H, c=RP)
    return v[:, :, :, :H]


def build_nc(n_imgs, mm_dt=mybir.dt.float32r, enable_asserts=False):
    ng = n_imgs // GROUP
    nc = bacc.Bacc(
        "TRN2",
        target_bir_lowering=False,
        debug=False,
        enable_asserts=enable_asserts,
    )
    DT = mm_dt
    x_d = nc.dram_tensor("x", (n_imgs, CIN, H, H), DT, kind="ExternalInput")
    w0_d = nc.dram_tensor("w0T", (CIN, 3, DM), DT, kind="ExternalInput")
    wd_d = nc.dram_tensor("wdT", (NRES, 3, DM, DM), DT, kind="ExternalInput")
    w1_d = nc.dram_tensor("w1T", (NRES, DM, DM), DT, kind="ExternalInput")
    c0_d = nc.dram_tensor("c0wT", (2, DM, DM), DT, kind="ExternalInput")
    wf_d = nc.dram_tensor("wfT", (DM, DM), DT, kind="ExternalInput")
    b_d = nc.dram_tensor("biases", (DM, 12), F32, kind="ExternalInput")
    out_d = nc.dram_tensor("out", (n_imgs, 4, DOUT, H, H), F32, kind="ExternalOutput")

    with tile.TileContext(nc) as tc:
        with (
            tc.tile_pool(name="consts", bufs=1) as consts,
            tc.tile_pool(name="tmp", bufs=3) as tmp,
            tc.tile_pool(name="outp", bufs=3) as outp,
            tc.tile_pool(name="psp", bufs=4, space="PSUM") as psp,
        ):
            w0_sb = consts.tile([CIN, 3, DM], DT, tag="w0")
            wd_sb = consts.tile([DM, NRES, 3, DM], DT, tag="wd")
            w1_sb = consts.tile([DM, NRES, DM], DT, tag="w1")
            c0_sb = consts.tile([DM, 2, DM], DT, tag="c0")
            wf_sb = consts.tile([DM, DM], DT, tag="wf")
            bias_sb = consts.tile([DM, 12], F32, tag="bias")
            nc.sync.dma_start(out=w0_sb, in_=w0_d.ap())
            nc.sync.dma_start(out=wd_sb, in_=wd_d.ap().rearrange("l k i o -> i l k o"))
            nc.sync.dma_start(out=w1_sb, in_=w1_d.ap().rearrange("l i o -> i l o"))
            nc.sync.dma_start(out=c0_sb, in_=c0_d.ap().rearrange("t i o -> i t o"))
            nc.sync.dma_start(out=wf_sb, in_=wf_d.ap())
            nc.sync.dma_start(out=bias_sb, in_=b_d.ap())

            # persistent double-buffered activation state; pad cols/rows and
            # guards are zeroed once here and only 15x15 data interiors are
            # written afterwards, preserving conv zero-padding semantics.
            xs_bufs = [
                consts.tile([DM, 4, XLEN], DT, tag=f"xs{i}", name=f"xs{i}")
                for i in range(3)
            ]
            xp_bufs = [
                consts.tile([CIN, XLEN], DT, tag=f"xp{i}", name=f"xp{i}")
                for i in range(3)
            ]
            # zero-fill via uint32 bitcast: walrus has no f32r memset
            # encoding, and 0 is bit-identical across formats.
            for t in xs_bufs:
                nc.vector.memset(t.bitcast(mybir.dt.uint32), 0)
            for t in xp_bufs:
                nc.vector.memset(t.bitcast(mybir.dt.uint32), 0)

            x_v = x_d.ap().rearrange("b c h w -> c b h w")
            out_v = out_d.ap().rearrange("b d o h w -> o d b (h w)")

            LANES = min(3, max(ng, 1))

            def group_stages(g):
                """Yield stage closures; each stage = paired PSUM fill+drain
                over 2-bank tiles (one direction pair each) so 4 PSUM slots
                pipeline across the 3 lanes."""
                xs = xs_bufs[g % LANES]
                xp = xp_bufs[g % LANES]
                i0 = g * GROUP

                def pair_tiles(nm):
                    return [
                        psp.tile([DM, 2, NB], F32, tag="ps", name=f"{nm}{g}p{p}")
                        for p in range(2)
                    ]

                def s_init():
                    for i in range(GROUP):
                        dst = xp[:, GUARD + i * IMGB: GUARD + (i + 1) * IMGB]
                        dst = dst.rearrange("p (r c) -> p r c", r=RP, c=RP)
                        nc.sync.dma_start(out=dst[:, :H, :H], in_=x_v[:, i0 + i])
                    ps = pair_tiles("psI")
                    for p in range(2):
                        for k in range(3):
                            for dd in range(2):
                                d = 2 * p + dd
                                off = (k - 1) * OFFS[d]
                                v = xp[:, GUARD + off: GUARD + off + PAY]
                                v = v.rearrange(
                                    "p (i r c) -> p i r c", i=GROUP, r=RP, c=RP
                                )
                                nc.tensor.matmul(
                                    ps[p][:, dd, :NMM], w0_sb[:, k, :],
                                    v[:, :, :H, :],
                                    start=(k == 0), stop=(k == 2),
                                )
                    for d in range(4):
                        nc.scalar.activation(
                            _int15(xs, d), _strip(ps[d // 2], d % 2), SILU,
                            bias=bias_sb[:, 0:1],
                        )
                yield s_init

                for l in range(NRES):
                    def s_taps(l=l):
                        ps = pair_tiles(f"psA{l}_")
                        for p in range(2):
                            for k in range(3):
                                for dd in range(2):
                                    d = 2 * p + dd
                                    off = (k - 1) * OFFS[d]
                                    nc.tensor.matmul(
                                        ps[p][:, dd, :NMM], wd_sb[:, l, k, :],
                                        _rhs(xs, d, off),
                                        start=(k == 0), stop=(k == 2),
                                    )
                        t1 = tmp.tile([DM, 4, NMM], DT, tag="t1", name=f"t1_{g}_{l}")
                        for p in range(2):
                            nc.scalar.activation(
                                t1[:, 2 * p: 2 * p + 2, :], ps[p][:, :, :NMM],
                                SILU, bias=bias_sb[:, 1 + l: 2 + l],
                            )
                        stage_out[0] = t1
                    def s_mix(l=l):
                        t1 = stage_out[0]
                        ps = pair_tiles(f"psB{l}_")
                        for p in range(2):
                            for dd in range(2):
                                d = 2 * p + dd
                                nc.tensor.matmul(
                                    ps[p][:, dd, :NMM], w1_sb[:, l, :],
                                    t1[:, d, :], start=True, stop=True,
                                )
                        t2 = tmp.tile([DM, 4, NMM], DT, tag="t2", name=f"t2_{g}_{l}")
                        for p in range(2):
                            nc.scalar.activation(
                                t2[:, 2 * p: 2 * p + 2, :], ps[p][:, :, :NMM],
                                SILU, bias=bias_sb[:, 5 + l: 6 + l],
                            )
                        for d in range(4):
                            xi = _int15(xs, d)
                            nc.vector.tensor_add(xi, xi, _strip(t2, d))
                    stage_out = [None]
                    yield s_taps
                    yield s_mix

                def s_c0a():
                    ps = pair_tiles("psC")
                    for p in range(2):
                        for dd in range(2):
                            d = 2 * p + dd
                            nc.tensor.matmul(
                                ps[p][:, dd, :NMM], c0_sb[:, 0, :], _w240(xs, d),
                                start=True, stop=True,
                            )
                    u1 = tmp.tile([DM, 4, NMM], DT, tag="u1", name=f"u1_{g}")
                    for p in range(2):
                        nc.scalar.activation(
                            u1[:, 2 * p: 2 * p + 2, :], ps[p][:, :, :NMM],
                            SILU, bias=bias_sb[:, 9:10],
                        )
                    c0_out[0] = u1
                c0_out = [None]
                yield s_c0a

                def s_c0b():
                    u1 = c0_out[0]
                    ps = pair_tiles("psD")
                    for p in range(2):
                        for dd in range(2):
                            d = 2 * p + dd
                            nc.tensor.matmul(
                                ps[p][:, dd, :NMM], c0_sb[:, 1, :], u1[:, d, :],
                                start=True, stop=True,
                            )
                    u2 = tmp.tile([DM, 4, NMM], DT, tag="u2", name=f"u2_{g}")
                    for p in range(2):
                        nc.scalar.activation(
                            u2[:, 2 * p: 2 * p + 2, :], ps[p][:, :, :NMM],
                            SILU, bias=bias_sb[:, 10:11],
                        )
                    for d in range(4):
                        u2d = u2[:, d, :].rearrange("p (i q) -> p i q", i=GROUP)
                        nc.vector.tensor_add(u2d, u2d, _w240(xs, d))
                    c0_out[0] = u2
                yield s_c0b

                def s_final():
                    u2 = c0_out[0]
                    ps = pair_tiles("psF")
                    for p in range(2):
                        for dd in range(2):
                            d = 2 * p + dd
                            nc.tensor.matmul(
                                ps[p][:, dd, :NMM], wf_sb, u2[:, d, :],
                                start=True, stop=True,
                            )
                    ob = outp.tile([DOUT, 4, NOUT], F32, tag="ob", name=f"ob{g}")
                    for d in range(4):
                        ob_d = ob[:, d, :].rearrange(
                            "p (i r c) -> p i r c", i=GROUP, r=H, c=H
                        )
                        nc.vector.tensor_scalar_add(
                            ob_d, _strip(ps[d // 2][:DOUT, :, :], d % 2),
                            bias_sb[:DOUT, 11:12],
                        )
                    ob_v = ob.rearrange("o d (i p) -> o d i p", i=GROUP)
                    for i in range(GROUP):
                        nc.sync.dma_start(
                            out=out_v[:, :, i0 + i, :], in_=ob_v[:, :, i, :]
                        )
                yield s_final

            # 3-lane software pipeline: emit stages round-robin across
            # lanes so the scheduler always has another group's PSUM
            # fill/drain to overlap with this one's serial chain.
            streams = [None] * LANES
            lane_groups = [[] for _ in range(LANES)]
            for g in range(ng):
                lane_groups[g % LANES].append(g)

            def lane_stream(groups):
                for g in groups:
                    yield from group_stages(g)

            streams = [lane_stream(gs) for gs in lane_groups]
            # Skew the lanes by ~1/3 of a group's 12 stages so the
            # ACT-less tail stages (adds/final/DMA) of different groups
            # never align — otherwise ScalarE starves periodically and
            # the PE p-state drops.
            for li, s in enumerate(streams):
                prime = 4 * (LANES - 1 - li)
                for _ in range(prime):
                    stage = next(s, None)
                    if stage is not None:
                        stage()
            while streams:
                nxt = []
                for s in streams:
                    stage = next(s, None)
                    if stage is not None:
                        stage()
                        nxt.append(s)
                streams = nxt

    nc.compile()
    return nc


def prep_weights(w_d0, res_wd, res_w1, c0_w1, c0_w2, wf,
                 b_d0, res_bd, res_b1, c0_b1, c0_b2, bf):
    f = lambda a: np.ascontiguousarray(np.asarray(a), dtype=np.float32)
    w0T = f(np.asarray(w_d0).transpose(1, 2, 0))          # [ci, k, co]
    wdT = f(np.asarray(res_wd).transpose(0, 3, 2, 1))     # [l, k, ci, co]
    w1T = f(np.asarray(res_w1).transpose(0, 2, 1))        # [l, ci, co]
    c0wT = f(np.stack([np.asarray(c0_w1).T, np.asarray(c0_w2).T]))
    wfT = np.zeros((DM, DM), np.float32)                  # pad 64 -> 128 cols
    wfT[:, :DOUT] = np.asarray(wf).T
    biases = np.zeros((DM, 12), np.float32)
    biases[:, 0] = np.asarray(b_d0)
    for l in range(NRES):
        biases[:, 1 + l] = np.asarray(res_bd)[l]
        biases[:, 5 + l] = np.asarray(res_b1)[l]
    biases[:, 9] = np.asarray(c0_b1)
    biases[:, 10] = np.asarray(c0_b2)
    biases[:DOUT, 11] = np.asarray(bf)
    return dict(w0T=w0T, wdT=wdT, w1T=w1T, c0wT=c0wT, wfT=wfT, biases=biases)


_NC_CACHE = {}


def _get_nc():
    if "nc" not in _NC_CACHE:
        _NC_CACHE["nc"] = build_nc(BPC)
    return _NC_CACHE["nc"]


def kernel(x, w_d0, b_d0, res_wd, res_bd, res_w1, res_b1,
           c0_w1, c0_b1, c0_w2, c0_b2, wf, bf, _trace=False):
    x = np.ascontiguousarray(np.asarray(x), dtype=np.float32)
    assert x.shape == (BATCH, CIN, H, H), x.shape
    w = prep_weights(w_d0, res_wd, res_w1, c0_w1, c0_w2, wf,
                     b_d0, res_bd, res_b1, c0_b1, c0_b2, bf)
    nc = _get_nc()
    in_maps = [
        dict(x=np.ascontiguousarray(x[c * BPC:(c + 1) * BPC]), **w)
        for c in range(NCORES)
    ]
    res = bass_utils.run_bass_kernel_spmd(
        nc, in_maps, core_ids=list(range(NCORES)), trace=_trace
    )
    out = np.concatenate([r["out"] for r in res.results], axis=0)
    if _trace:
        return out, res
    return out

